# revision 29
# baseline (speedup 1.0000x reference)
"""BigBird attention (B=4, N=4096, D=1024, H=16, BS=64) on 8 TRN2 NeuronCores.

Sharding: batch (4-way) x head-group (2-way).  Core c handles batch c//2 and
heads [hg*8, hg*8+8) where hg = c%2 (d_model slice [hg*512, hg*512+512)).

Per core:
  pass A: QKV projections.  x.T tiles produced with DMA transposes; q/k
          emitted transposed (qT/kT: [dl, n], head dim on partitions), v
          natural.  score scale folded into Wq/bq on the host; bv dropped
          entirely (attention is affine in v: host adds c(q)*bv@Wo.T).
  pass B: per-head BigBird attention, all scores computed transposed
          (S^T = K_chunk^T Q, keys on partitions) so probabilities feed the
          AV matmuls directly as stationary operands -- no PE transposes.
          The sliding-window mask is folded into 4 extra contraction rows
          (rank-2 outer product of periodic 0/1 q-patterns and -1e9
          k-patterns), so exp() yields exact zeros in the masked corners.
          No max subtraction (scores bounded ~|3|).  V carries a ones
          column so each AV matmul also emits the softmax denominator
          per-partition; normalization is a per-partition reciprocal.
  pass C: transpose ctx with the PE, then row-parallel output projection
          -> partial outT [d_model, n] (f32).
Host combines: out[b] = outT(core 2b).T + outT(core 2b+1).T + bo + c(q)*bv@Wo.T
with c(q) = 1 for rows in global blocks else 2.

The kernel is specialized (compiled) per global_indices value.
"""

import functools
import sys

import numpy as np

P = 128
BS = 64
NEG = -1e9


def _ensure_path():
    try:
        import concourse.bass  # noqa: F401
    except ImportError:
        sys.path.insert(0, "/opt/trn_rl_repo")


def _build(n, dmodel, dl, g0, g1, dbg=0):
    """Build the per-core Bass program.

    n: sequence length per core, dmodel: model dim, dl: local head dims =
    hpc*64.  g0, g1: global block indices (compile-time constants).
    """
    _ensure_path()
    from contextlib import ExitStack

    import concourse.bass as bass  # noqa: F401
    import concourse.tile as tile
    from concourse import bacc, mybir
    from concourse.masks import make_identity

    f32 = mybir.dt.float32
    bf16 = mybir.dt.bfloat16
    AF = mybir.ActivationFunctionType
    OP = mybir.AluOpType

    nch = n // 512     # 512-column chunks of the sequence
    ndc = dmodel // P  # contraction chunks for QKV proj
    njt = dl // P      # row tiles of qT/kT
    hpc = dl // BS     # heads per core
    nt = n // P        # query tiles (2 blocks each)
    nkc = nt + 1       # padded key chunks (128 keys each, shifted by -BS)
    ndc2 = dl // P     # contraction chunks for out proj

    nc = bacc.Bacc(None, target_bir_lowering=False, debug=False)

    xT_d = nc.dram_tensor("xT", [dmodel, n], bf16, kind="ExternalInput")
    wq_d = nc.dram_tensor("wqT", [dmodel, dl], bf16, kind="ExternalInput")
    wk_d = nc.dram_tensor("wkT", [dmodel, dl], bf16, kind="ExternalInput")
    wv_d = nc.dram_tensor("wvT", [dmodel, dl], bf16, kind="ExternalInput")
    wo_d = nc.dram_tensor("woT", [dl, dmodel], bf16, kind="ExternalInput")
    bq_d = nc.dram_tensor("bq", [dl], f32, kind="ExternalInput")
    bk_d = nc.dram_tensor("bk", [dl], f32, kind="ExternalInput")
    qm_d = nc.dram_tensor("qmask", [64, n], bf16, kind="ExternalInput")
    km_d = nc.dram_tensor("kmask", [64, n + 2 * BS], bf16, kind="ExternalInput")
    out_d = nc.dram_tensor("outT", [dmodel, n], bf16, kind="ExternalOutput")
    if dbg:
        qTo_d = nc.dram_tensor("qTo", [dl, n], bf16, kind="ExternalOutput")
        kTo_d = nc.dram_tensor("kTo", [dl, n], bf16, kind="ExternalOutput")
        vo_d = nc.dram_tensor("vo", [n, dl], bf16, kind="ExternalOutput")
        ctxo_d = nc.dram_tensor("ctxo", [P, n // P, dl], bf16, kind="ExternalOutput")

    with tile.TileContext(nc) as tc, ExitStack() as top:
        dram = top.enter_context(tc.tile_pool(name="dram", bufs=1, space="DRAM"))
        qT_d = dram.tile([dl, n], bf16)
        kT_d = dram.tile([dl, n], bf16)
        v_d = dram.tile([n, dl], bf16)

        const = top.enter_context(tc.tile_pool(name="const", bufs=1))
        ident = const.tile([P, P], bf16)
        make_identity(nc, ident)

        # ctx natural accumulator: [q mod 128, tile, head*64+dh], SBUF-resident
        ctx_pool = top.enter_context(tc.tile_pool(name="ctx", bufs=1))
        ctx_nat = ctx_pool.tile([P, nt, dl], bf16)

        # pass-B per-head slots (manual ping-pong).  Allocated at top level so
        # their memory is disjoint from the pass-A pools: the constant regions
        # (mask rows, ones columns) are written once, up front.
        slot = top.enter_context(tc.tile_pool(name="slot", bufs=1))
        qz_s = [slot.tile([P, n], bf16, tag=f"qz{i}", name=f"qz{i}") for i in range(2)]
        kp_s = [slot.tile([P, n + 2 * BS], bf16, tag=f"kp{i}", name=f"kp{i}") for i in range(2)]
        va_s = [slot.tile([P, nkc, BS + 1], bf16, tag=f"va{i}", name=f"va{i}") for i in range(2)]
        kg_s = [slot.tile([P, P], bf16, tag=f"kg{i}", name=f"kg{i}") for i in range(2)]
        vg_s = [slot.tile([P, BS + 1], bf16, tag=f"vg{i}", name=f"vg{i}") for i in range(2)]
        qg_s = [slot.tile([P, P], bf16, tag=f"qg{i}", name=f"qg{i}") for i in range(2)]
        def init_slot_consts():
            for qz in qz_s:
                nc.sync.dma_start(qz[64:P, :], qm_d[:, :])
            for kp in kp_s:
                nc.sync.dma_start(kp[64:P, :], km_d[:, :])
            for kg in kg_s:
                nc.gpsimd.memset(kg[64:P, :], 0.0)
            for qg in qg_s:
                nc.gpsimd.memset(qg[64:P, :], 0.0)
            for va in va_s:
                nc.gpsimd.memset(va[:, :, BS : BS + 1], 1.0)
            for vg in vg_s:
                nc.gpsimd.memset(vg[:, BS : BS + 1], 1.0)

        # ---------------- pass A: projections ----------------
        with ExitStack() as ps:
            wpool = ps.enter_context(tc.tile_pool(name="wpool", bufs=1))
            wq_sb = wpool.tile([P, ndc, dl], bf16)
            wk_sb = wpool.tile([P, ndc, dl], bf16)
            wv_sb = wpool.tile([P, ndc, dl], bf16)
            psA = ps.enter_context(tc.tile_pool(name="psA", bufs=4, space="PSUM"))
            xtpool = ps.enter_context(tc.tile_pool(name="xtpool", bufs=3))
            aout = ps.enter_context(tc.tile_pool(name="aout", bufs=4))

            def load_xt(ch):
                n0 = ch * 512
                xT = xtpool.tile([P, ndc, 512], bf16, tag="xT", name="xT")
                for dc in range(ndc):
                    nc.sync.dma_start(
                        xT[:, dc, :], xT_d[dc * P : (dc + 1) * P, n0 : n0 + 512]
                    )
                return xT

            # first x chunk ahead of the (big) weight loads: the sync queue is
            # in-order, and the first matmuls need xT(ch0) + wq[dc0] only.
            nc.sync.dma_start(wq_sb[:, 0, :], wq_d[0:P, :])
            xt_next = load_xt(0)
            for a in range(1, ndc):
                nc.sync.dma_start(wq_sb[:, a, :], wq_d[a * P : (a + 1) * P, :])
            nc.sync.dma_start(wk_sb, wk_d.rearrange("(a p) j -> p a j", p=P))
            nc.sync.dma_start(wv_sb, wv_d.rearrange("(a p) j -> p a j", p=P))
            bq_sb = wpool.tile([P, njt], f32)
            bk_sb = wpool.tile([P, njt], f32)
            nc.scalar.dma_start(bq_sb, bq_d.rearrange("(a p) -> p a", p=P))
            nc.scalar.dma_start(bk_sb, bk_d.rearrange("(a p) -> p a", p=P))

            for ch in range(nch):
                n0 = ch * 512
                xT = xt_next
                if ch + 1 < nch:
                    xt_next = load_xt(ch + 1)
                if ch == 2:
                    init_slot_consts()
                # qT / kT (transposed outputs, bias per-partition)
                for w_sb, b_sb, dst in ((wq_sb, bq_sb, qT_d), (wk_sb, bk_sb, kT_d)):
                    for jt in range(njt):
                        pp = psA.tile([P, 512], f32, tag="ps_a")
                        for dc in range(ndc):
                            nc.tensor.matmul(
                                pp,
                                w_sb[:, dc, jt * P : (jt + 1) * P],
                                xT[:, dc, :],
                                start=(dc == 0),
                                stop=(dc == ndc - 1),
                            )
                        ot = aout.tile([P, 512], bf16, tag="aout")
                        nc.scalar.activation(
                            ot, pp, AF.Identity, bias=b_sb[:, jt : jt + 1]
                        )
                        nc.scalar.dma_start(
                            dst[jt * P : (jt + 1) * P, n0 : n0 + 512], ot
                        )
                # v (natural layout, no bias -- folded to host)
                for ns in range(4):
                    pp = psA.tile([P, dl], f32, tag="ps_a")
                    for dc in range(ndc):
                        nc.tensor.matmul(
                            pp,
                            xT[:, dc, ns * P : (ns + 1) * P],
                            wv_sb[:, dc, :],
                            start=(dc == 0),
                            stop=(dc == ndc - 1),
                        )
                    ot = aout.tile([P, dl], bf16, tag="aout_v")
                    nc.scalar.copy(ot, pp)
                    nc.scalar.dma_start(v_d[n0 + ns * P : n0 + (ns + 1) * P, :], ot)

        # ---------------- pass B: attention ----------------
        with ExitStack() as ps:
            apool = ps.enter_context(tc.tile_pool(name="apool", bufs=4))
            agp = ps.enter_context(tc.tile_pool(name="agp", bufs=2))
            agr = ps.enter_context(tc.tile_pool(name="agr", bufs=8))
            stat = ps.enter_context(tc.tile_pool(name="stat", bufs=4))
            tgp = ps.enter_context(tc.tile_pool(name="tgp", bufs=4))
            psS = ps.enter_context(tc.tile_pool(name="psS", bufs=3, space="PSUM"))
            psC = ps.enter_context(tc.tile_pool(name="psC", bufs=4, space="PSUM"))
            psQ = ps.enter_context(tc.tile_pool(name="psQ", bufs=1, space="PSUM"))

            p0s = (g0 % 2) * BS
            p1s = (g1 % 2) * BS

            for h in range(hpc):
                r0 = h * BS
                qz, kp, va = qz_s[h % 2], kp_s[h % 2], va_s[h % 2]
                kg, vg, qg = kg_s[h % 2], vg_s[h % 2], qg_s[h % 2]

                # -- per-head DMAs (overlap previous head's compute) --
                h2 = n // 2
                nc.sync.dma_start(kp[0:64, 0:BS], kT_d[r0 : r0 + BS, n - BS : n])
                nc.sync.dma_start(
                    kp[0:64, BS : BS + h2], kT_d[r0 : r0 + BS, 0:h2]
                )
                nc.sync.dma_start(qz[0:64, 0:h2], qT_d[r0 : r0 + BS, 0:h2])
                nc.sync.dma_start(
                    kp[0:64, BS + h2 : BS + n], kT_d[r0 : r0 + BS, h2:n]
                )
                nc.sync.dma_start(qz[0:64, h2:n], qT_d[r0 : r0 + BS, h2:n])
                nc.sync.dma_start(kp[0:64, BS + n :], kT_d[r0 : r0 + BS, 0:BS])
                vs = v_d[:, r0 : r0 + BS]
                nc.sync.dma_start(va[0:BS, 0, 0:BS], vs[n - BS : n, :])
                nc.sync.dma_start(va[BS:P, 0, 0:BS], vs[0:BS, :])
                nc.sync.dma_start(
                    va[:, 1 : nkc - 1, 0:BS],
                    vs[BS : n - BS, :].rearrange("(a p) c -> p a c", p=P),
                )
                nc.sync.dma_start(va[0:BS, nkc - 1, 0:BS], vs[n - BS : n, :])
                nc.sync.dma_start(va[BS:P, nkc - 1, 0:BS], vs[0:BS, :])
                for gi, gv in enumerate((g0, g1)):
                    nc.sync.dma_start(
                        kg[0:64, gi * BS : (gi + 1) * BS],
                        kT_d[r0 : r0 + BS, gv * BS : (gv + 1) * BS],
                    )
                    nc.sync.dma_start(
                        vg[gi * BS : (gi + 1) * BS, 0:BS],
                        vs[gv * BS : (gv + 1) * BS, :],
                    )
                    nc.sync.dma_start(
                        qg[0:64, gi * BS : (gi + 1) * BS],
                        qT_d[r0 : r0 + BS, gv * BS : (gv + 1) * BS],
                    )

                # -- local + global-col scores (S^T layout), exp, AV --
                def sc_pair(pr):
                    """scores+exp for padded key chunks 2pr, 2pr+1 (batched)."""
                    sps = psS.tile([P, 2, 256], f32, tag="sps")
                    a_sb = apool.tile([P, 2, 256], bf16, tag="a")
                    nws = []
                    for i in (0, 1):
                        c = 2 * pr + i
                        if c > nt:
                            continue
                        lo = max(0, (c - 1)) * P
                        hi = min(nt, c + 1) * P
                        nws.append(hi - lo)
                        nc.tensor.matmul(
                            sps[:, i, 0 : hi - lo],
                            kp[:, c * P : (c + 1) * P],
                            qz[:, lo:hi],
                            start=True,
                            stop=True,
                        )
                    if nws == [256, 256]:
                        nc.scalar.activation(a_sb, sps, AF.Exp)
                    else:
                        for i, nw in enumerate(nws):
                            nc.scalar.activation(
                                a_sb[:, i, 0:nw], sps[:, i, 0:nw], AF.Exp
                            )
                    return a_sb

                def gc_group(j):
                    spg = psS.tile([P, 512], f32, tag="sps")
                    nc.tensor.matmul(
                        spg, kg, qz[:, j * 512 : (j + 1) * 512], start=True, stop=True
                    )
                    ag = agp.tile([P, 512], bf16, tag="ag")
                    nc.scalar.activation(ag, spg, AF.Exp)
                    return ag

                ag_nxt = gc_group(0)
                a_pair = {0: sc_pair(0), 1: sc_pair(1)}
                ag_cur = None
                cps2 = None
                for t in range(nt):
                    if t % 4 == 0:
                        ag_cur = ag_nxt
                    if t % 4 == 1 and t // 4 + 1 < 8:
                        ag_nxt = gc_group(t // 4 + 1)
                    want = min(nt // 2, t // 2 + 2)
                    if want not in a_pair:
                        a_pair[want] = sc_pair(want)
                        a_pair.pop(want - 3, None)
                    a_lo = a_pair[t // 2][:, t % 2, :]
                    off = 0 if t == 0 else P
                    a_up = a_pair[(t + 1) // 2][:, (t + 1) % 2, :]
                    if t % 2 == 0:
                        cps2 = psC.tile([P, 260], f32, tag="cps")
                    co = (t % 2) * 130
                    cps = cps2[:, co : co + 130]
                    nc.tensor.matmul(
                        cps[:, 0:65],
                        a_lo[:, off : off + P],
                        va[:, t, :],
                        start=True,
                        stop=False,
                    )
                    nc.tensor.matmul(
                        cps[:, 0:65],
                        a_up[:, 0:P],
                        va[:, t + 1, :],
                        start=False,
                        stop=True,
                    )
                    nc.tensor.matmul(
                        cps[:, 65:130],
                        ag_cur[:, (t % 4) * P : (t % 4 + 1) * P],
                        vg,
                        start=True,
                        stop=True,
                    )
                    if t % 2 == 0:
                        continue
                    # batched per-partition reciprocals for both tiles
                    r4 = stat.tile([P, 4], f32, tag="r4")
                    nc.vector.reciprocal(r4, cps2[:, 64:260:65])
                    for tt, cc, ri in ((t - 1, 0, 0), (t, 130, 2)):
                        tg = tgp.tile([P, BS], f32, tag="tg")
                        nc.vector.tensor_scalar_mul(
                            tg, cps2[:, cc + 65 : cc + 129], r4[:, ri + 1 : ri + 2]
                        )
                        nc.vector.scalar_tensor_tensor(
                            ctx_nat[:, tt, r0 : r0 + BS],
                            cps2[:, cc : cc + 64],
                            r4[:, ri : ri + 1],
                            tg,
                            OP.mult,
                            OP.add,
                        )

                # -- global rows: full attention for the 2 global q blocks --
                cpr0 = psQ.tile([P, 130], f32, tag="cpr0")
                cpr1 = cpr0

                def grow_scores(j):
                    spr = psS.tile([P, 4, P], f32, tag="sps")
                    for i in range(4):
                        c = 1 + 4 * j + i
                        nc.tensor.matmul(
                            spr[:, i, :],
                            kp[:, c * P : (c + 1) * P],
                            qg,
                            start=True,
                            stop=True,
                        )
                    ar = agr.tile([P, 4, P], bf16, tag="ar")
                    nc.scalar.activation(ar, spr, AF.Exp)
                    return ar

                ars = [grow_scores(0)]
                for j in range(8):
                    if j + 1 < 8:
                        ars.append(grow_scores(j + 1))
                    for i in range(4):
                        c = 1 + 4 * j + i
                        nc.tensor.matmul(
                            cpr0[p0s : p0s + BS, 0:65],
                            ars[j][:, i, 0:BS],
                            va[:, c, :],
                            start=(c == 1),
                            stop=(c == nkc - 1),
                        )
                for j in range(8):
                    for i in range(4):
                        c = 1 + 4 * j + i
                        nc.tensor.matmul(
                            cpr1[p1s : p1s + BS, 65:130],
                            ars[j][:, i, BS:P],
                            va[:, c, :],
                            start=(c == 1),
                            stop=(c == nkc - 1),
                        )
                for gi, gv, pb, co in ((0, g0, p0s, 0), (1, g1, p1s, 65)):
                    rg = stat.tile([P, 1], f32, tag=f"rg{gi}")
                    nc.vector.reciprocal(
                        rg[pb : pb + BS, :], cpr0[pb : pb + BS, co + 64 : co + 65]
                    )
                    nc.vector.tensor_scalar_mul(
                        ctx_nat[pb : pb + BS, gv // 2, r0 : r0 + BS],
                        cpr0[pb : pb + BS, co : co + 64],
                        rg[pb : pb + BS, :],
                    )

        # ---------------- pass C: output projection ----------------
        with ExitStack() as ps:
            wop = ps.enter_context(tc.tile_pool(name="wop", bufs=1))
            wo_sb = wop.tile([P, ndc2, dmodel], bf16)
            nc.sync.dma_start(wo_sb, wo_d.rearrange("(a p) o -> p a o", p=P))
            ctp = ps.enter_context(tc.tile_pool(name="ctp", bufs=2))
            copool = ps.enter_context(tc.tile_pool(name="co", bufs=4))
            psT = ps.enter_context(tc.tile_pool(name="psT", bufs=4, space="PSUM"))
            psO = ps.enter_context(tc.tile_pool(name="psO", bufs=4, space="PSUM"))
            for ncc in range(nch):
                ctxT = ctp.tile([P, ndc2, 512], bf16, tag="ctxT")
                for ti in range(4):
                    t = ncc * 4 + ti
                    for dc in range(ndc2):
                        tp = psT.tile([P, P], bf16, tag="tp")
                        nc.tensor.transpose(
                            tp, ctx_nat[:, t, dc * P : (dc + 1) * P], ident
                        )
                        if dc % 2 == 0:
                            nc.scalar.copy(ctxT[:, dc, ti * P : (ti + 1) * P], tp)
                        else:
                            nc.vector.tensor_copy(ctxT[:, dc, ti * P : (ti + 1) * P], tp)
                for ot in range(dmodel // P):
                    pp = psO.tile([P, 512], f32, tag="pso")
                    for dc in range(ndc2):
                        nc.tensor.matmul(
                            pp,
                            wo_sb[:, dc, ot * P : (ot + 1) * P],
                            ctxT[:, dc, :],
                            start=(dc == 0),
                            stop=(dc == ndc2 - 1),
                        )
                    ob = copool.tile([P, 512], bf16, tag="ob")
                    if ot % 2 == 0:
                        nc.scalar.copy(ob, pp)
                    else:
                        nc.vector.tensor_copy(ob, pp)
                    nc.sync.dma_start(
                        out_d[ot * P : (ot + 1) * P, ncc * 512 : (ncc + 1) * 512], ob
                    )

        if dbg:
            nc.sync.dma_start(qTo_d[:, :], qT_d)
            nc.sync.dma_start(kTo_d[:, :], kT_d)
            nc.sync.dma_start(vo_d[:, :], v_d)
            nc.sync.dma_start(ctxo_d[:, :, :], ctx_nat)

    nc.finalize()
    return nc


@functools.lru_cache(maxsize=8)
def _get(n, dmodel, dl, g0, g1):
    return _build(n, dmodel, dl, g0, g1)


def _prepare(inputs):
    """Build (nc, in_maps, meta) for the SPMD run from full unsharded inputs."""
    x = np.asarray(inputs["x"], np.float32)
    Wq = np.asarray(inputs["Wq"], np.float32)
    Wk = np.asarray(inputs["Wk"], np.float32)
    Wv = np.asarray(inputs["Wv"], np.float32)
    Wo = np.asarray(inputs["Wo"], np.float32)
    bq = np.asarray(inputs["bq"], np.float32)
    bk = np.asarray(inputs["bk"], np.float32)
    bv = np.asarray(inputs["bv"], np.float32)
    bo = np.asarray(inputs["bo"], np.float32)
    gi = np.asarray(inputs["global_indices"]).astype(np.int64)
    g0, g1 = int(gi[0]), int(gi[1])

    b_, n_, d_ = x.shape
    dl = d_ // 2
    scale = 1.0 / np.sqrt(np.float32(64.0)).astype(np.float32)

    nc = _get(n_, d_, dl, g0, g1)

    import ml_dtypes

    bf = ml_dtypes.bfloat16
    # mask pattern rows (periodic in the column index, see pass B docstring)
    NEGf = np.float32(-1e9)
    j = np.arange(n_) % 256
    qmask = np.zeros((64, n_), np.float32)
    qmask[0, (j >= 128) & (j < 192)] = 1.0  # w1e
    qmask[1, (j >= 64) & (j < 128)] = 1.0   # w2e
    qmask[2, j < 64] = 1.0                  # w1o
    qmask[3, j >= 192] = 1.0                # w2o
    qmask = np.ascontiguousarray(qmask).astype(bf)
    jk = np.arange(n_ + 128) % 256
    kmask = np.zeros((64, n_ + 128), np.float32)
    kmask[0, (jk >= 64) & (jk < 128)] = NEGf   # u1e
    kmask[1, jk < 64] = NEGf                   # u2e
    kmask[2, jk >= 192] = NEGf                 # u1o
    kmask[3, (jk >= 128) & (jk < 192)] = NEGf  # u2o
    kmask = np.ascontiguousarray(kmask).astype(bf)
    in_maps = []
    for c in range(8):
        b, hg = divmod(c, 2)
        S = slice(hg * dl, (hg + 1) * dl)
        in_maps.append(
            {
                "xT": np.ascontiguousarray(x[b].T).astype(bf),
                "qmask": qmask,
                "kmask": kmask,
                "wqT": np.ascontiguousarray((Wq[S, :] * scale).T).astype(bf),
                "wkT": np.ascontiguousarray(Wk[S, :].T).astype(bf),
                "wvT": np.ascontiguousarray(Wv[S, :].T).astype(bf),
                "woT": np.ascontiguousarray(Wo[:, S].T).astype(bf),
                "bq": np.ascontiguousarray(bq[S] * scale),
                "bk": np.ascontiguousarray(bk[S]),
            }
        )

    # host-side bv correction: out[q] += c(q) * bv @ Wo.T, c(q)=1 on global
    # blocks (overwritten by full-attention rows), else 2.
    bvWo = bv @ Wo.T  # [d_model]
    coef = np.full((n_, 1), 2.0, np.float32)
    bs = 64
    coef[g0 * bs : (g0 + 1) * bs] = 1.0
    coef[g1 * bs : (g1 + 1) * bs] = 1.0
    corr = (coef * bvWo[None, :] + bo[None, :]).astype(np.float32)

    return nc, in_maps, (b_, n_, d_, corr)


def _combine(res, meta):
    b_, n_, d_, corr = meta
    out = np.empty((b_, n_, d_), np.float32)
    for b in range(b_):
        out[b] = (
            res[2 * b]["outT"].T.astype(np.float32)
            + res[2 * b + 1]["outT"].T.astype(np.float32)
            + corr
        )
    return out


def kernel(**inputs):
    _ensure_path()
    from concourse.bass_utils import run_bass_kernel_spmd

    nc, in_maps, meta = _prepare(inputs)
    res = run_bass_kernel_spmd(nc, in_maps, list(range(8))).results
    return _combine(res, meta)


# revision 30
# speedup vs baseline: 1.1884x; 1.1884x over previous
"""BigBird attention (B=4, N=4096, D=1024, H=16, BS=64) on 8 TRN2 NeuronCores.

Sharding: batch (4-way) x head-group (2-way).  Core c handles batch c//2 and
heads [hg*8, hg*8+8) where hg = c%2 (d_model slice [hg*512, hg*512+512)).

Per core:
  pass A: QKV projections.  x.T tiles produced with DMA transposes; q/k
          emitted transposed (qT/kT: [dl, n], head dim on partitions), v
          natural.  score scale folded into Wq/bq on the host; bv dropped
          entirely (attention is affine in v: host adds c(q)*bv@Wo.T).
  pass B: per-head BigBird attention, all scores computed transposed
          (S^T = K_chunk^T Q, keys on partitions) so probabilities feed the
          AV matmuls directly as stationary operands -- no PE transposes.
          The sliding-window mask is folded into 4 extra contraction rows
          (rank-2 outer product of periodic 0/1 q-patterns and -1e9
          k-patterns), so exp() yields exact zeros in the masked corners.
          No max subtraction (scores bounded ~|3|).  V carries a ones
          column so each AV matmul also emits the softmax denominator
          per-partition; normalization is a per-partition reciprocal.
  pass C: transpose ctx with the PE, then row-parallel output projection
          -> partial outT [d_model, n] (f32).
Host combines: out[b] = outT(core 2b).T + outT(core 2b+1).T + bo + c(q)*bv@Wo.T
with c(q) = 1 for rows in global blocks else 2.

The kernel is specialized (compiled) per global_indices value.
"""

import functools
import sys

import numpy as np

P = 128
BS = 64
NEG = -1e9


def _ensure_path():
    try:
        import concourse.bass  # noqa: F401
    except ImportError:
        sys.path.insert(0, "/opt/trn_rl_repo")


def _build(n, dmodel, dl, g0, g1, dbg=0):
    """Build the per-core Bass program.

    n: sequence length per core, dmodel: model dim, dl: local head dims =
    hpc*64.  g0, g1: global block indices (compile-time constants).
    """
    _ensure_path()
    from contextlib import ExitStack

    import concourse.bass as bass  # noqa: F401
    import concourse.tile as tile
    from concourse import bacc, mybir
    from concourse.masks import make_identity

    f32 = mybir.dt.float32
    bf16 = mybir.dt.bfloat16
    AF = mybir.ActivationFunctionType
    OP = mybir.AluOpType

    nch = n // 512     # 512-column chunks of the sequence
    ndc = dmodel // P  # contraction chunks for QKV proj
    njt = dl // P      # row tiles of qT/kT
    hpc = dl // BS     # heads per core
    nt = n // P        # query tiles (2 blocks each)
    nkc = nt + 1       # padded key chunks (128 keys each, shifted by -BS)
    ndc2 = dl // P     # contraction chunks for out proj

    nc = bacc.Bacc(None, target_bir_lowering=False, debug=False)

    xT_d = nc.dram_tensor("xT", [dmodel, n], bf16, kind="ExternalInput")
    wq_d = nc.dram_tensor("wqT", [dmodel, dl], bf16, kind="ExternalInput")
    wk_d = nc.dram_tensor("wkT", [dmodel, dl], bf16, kind="ExternalInput")
    wv_d = nc.dram_tensor("wvT", [dmodel, dl], bf16, kind="ExternalInput")
    wo_d = nc.dram_tensor("woT", [dl, dmodel], bf16, kind="ExternalInput")
    bq_d = nc.dram_tensor("bq", [dl], f32, kind="ExternalInput")
    bk_d = nc.dram_tensor("bk", [dl], f32, kind="ExternalInput")
    qm_d = nc.dram_tensor("qmask", [64, n], bf16, kind="ExternalInput")
    km_d = nc.dram_tensor("kmask", [64, n + 2 * BS], bf16, kind="ExternalInput")
    out_d = nc.dram_tensor("outT", [dmodel, n], bf16, kind="ExternalOutput")
    if dbg:
        qTo_d = nc.dram_tensor("qTo", [dl, n], bf16, kind="ExternalOutput")
        kTo_d = nc.dram_tensor("kTo", [dl, n], bf16, kind="ExternalOutput")
        vo_d = nc.dram_tensor("vo", [n, dl], bf16, kind="ExternalOutput")
        ctxo_d = nc.dram_tensor("ctxo", [P, n // P, dl], bf16, kind="ExternalOutput")

    with tile.TileContext(nc) as tc, ExitStack() as top:
        dram = top.enter_context(tc.tile_pool(name="dram", bufs=1, space="DRAM"))
        qT_d = dram.tile([dl, n], bf16)
        kT_d = dram.tile([dl, n], bf16)
        v_d = dram.tile([n, dl], bf16)

        const = top.enter_context(tc.tile_pool(name="const", bufs=1))
        ident = const.tile([P, P], bf16)
        make_identity(nc, ident)

        # ctx natural accumulator: [q mod 128, tile, head*64+dh], SBUF-resident
        ctx_pool = top.enter_context(tc.tile_pool(name="ctx", bufs=1))
        ctx_nat = ctx_pool.tile([P, nt, dl], bf16)

        # pass-B per-head slots (manual ping-pong).  Allocated at top level so
        # their memory is disjoint from the pass-A pools: the constant regions
        # (mask rows, ones columns) are written once, up front.
        slot = top.enter_context(tc.tile_pool(name="slot", bufs=1))
        qz_s = [slot.tile([P, n], bf16, tag=f"qz{i}", name=f"qz{i}") for i in range(2)]
        kp_s = [slot.tile([P, n + 2 * BS], bf16, tag=f"kp{i}", name=f"kp{i}") for i in range(2)]
        va_s = [slot.tile([P, nkc, BS + 1], bf16, tag=f"va{i}", name=f"va{i}") for i in range(2)]
        kg_s = [slot.tile([P, P], bf16, tag=f"kg{i}", name=f"kg{i}") for i in range(2)]
        vg_s = [slot.tile([P, BS + 1], bf16, tag=f"vg{i}", name=f"vg{i}") for i in range(2)]
        qg_s = [slot.tile([P, P], bf16, tag=f"qg{i}", name=f"qg{i}") for i in range(2)]
        def init_slot_consts():
            for qz in qz_s:
                nc.sync.dma_start(qz[64:P, :], qm_d[:, :])
            for kp in kp_s:
                nc.sync.dma_start(kp[64:P, :], km_d[:, :])
            for kg in kg_s:
                nc.gpsimd.memset(kg[64:P, :], 0.0)
            for qg in qg_s:
                nc.gpsimd.memset(qg[64:P, :], 0.0)
            for va in va_s:
                nc.gpsimd.memset(va[:, :, BS : BS + 1], 1.0)
            for vg in vg_s:
                nc.gpsimd.memset(vg[:, BS : BS + 1], 1.0)

        # ---------------- pass A: projections ----------------
        with ExitStack() as ps:
            wpool = ps.enter_context(tc.tile_pool(name="wpool", bufs=1))
            wq_sb = wpool.tile([P, ndc, dl], bf16)
            wk_sb = wpool.tile([P, ndc, dl], bf16)
            wv_sb = wpool.tile([P, ndc, dl], bf16)
            psA = ps.enter_context(tc.tile_pool(name="psA", bufs=4, space="PSUM"))
            xtpool = ps.enter_context(tc.tile_pool(name="xtpool", bufs=3))
            aout = ps.enter_context(tc.tile_pool(name="aout", bufs=4))

            def load_xt(ch):
                n0 = ch * 512
                xT = xtpool.tile([P, ndc, 512], bf16, tag="xT", name="xT")
                for dc in range(ndc):
                    nc.sync.dma_start(
                        xT[:, dc, :], xT_d[dc * P : (dc + 1) * P, n0 : n0 + 512]
                    )
                return xT

            # first x chunk ahead of the (big) weight loads: the sync queue is
            # in-order, and the first matmuls need xT(ch0) + wq[dc0] only.
            nc.sync.dma_start(wq_sb[:, 0, :], wq_d[0:P, :])
            xt_next = load_xt(0)
            for a in range(1, ndc):
                nc.sync.dma_start(wq_sb[:, a, :], wq_d[a * P : (a + 1) * P, :])
            nc.sync.dma_start(wk_sb, wk_d.rearrange("(a p) j -> p a j", p=P))
            nc.sync.dma_start(wv_sb, wv_d.rearrange("(a p) j -> p a j", p=P))
            bq_sb = wpool.tile([P, njt], f32)
            bk_sb = wpool.tile([P, njt], f32)
            nc.scalar.dma_start(bq_sb, bq_d.rearrange("(a p) -> p a", p=P))
            nc.scalar.dma_start(bk_sb, bk_d.rearrange("(a p) -> p a", p=P))

            for ch in range(nch):
                n0 = ch * 512
                xT = xt_next
                if ch + 1 < nch:
                    xt_next = load_xt(ch + 1)
                if ch == 2:
                    init_slot_consts()
                # qT / kT (transposed outputs, bias per-partition)
                for w_sb, b_sb, dst in ((wq_sb, bq_sb, qT_d), (wk_sb, bk_sb, kT_d)):
                    for jt in range(njt):
                        pp = psA.tile([P, 512], f32, tag="ps_a")
                        for dc in range(ndc):
                            nc.tensor.matmul(
                                pp,
                                w_sb[:, dc, jt * P : (jt + 1) * P],
                                xT[:, dc, :],
                                start=(dc == 0),
                                stop=(dc == ndc - 1),
                            )
                        ot = aout.tile([P, 512], bf16, tag="aout")
                        nc.scalar.activation(
                            ot, pp, AF.Identity, bias=b_sb[:, jt : jt + 1]
                        )
                        nc.scalar.dma_start(
                            dst[jt * P : (jt + 1) * P, n0 : n0 + 512], ot
                        )
                # v (natural layout, no bias -- folded to host)
                for ns in range(4):
                    pp = psA.tile([P, dl], f32, tag="ps_a")
                    for dc in range(ndc):
                        nc.tensor.matmul(
                            pp,
                            xT[:, dc, ns * P : (ns + 1) * P],
                            wv_sb[:, dc, :],
                            start=(dc == 0),
                            stop=(dc == ndc - 1),
                        )
                    ot = aout.tile([P, dl], bf16, tag="aout_v")
                    nc.scalar.copy(ot, pp)
                    nc.scalar.dma_start(v_d[n0 + ns * P : n0 + (ns + 1) * P, :], ot)

        # ---------------- pass B: attention ----------------
        with ExitStack() as ps:
            apool = ps.enter_context(tc.tile_pool(name="apool", bufs=4))
            agp = ps.enter_context(tc.tile_pool(name="agp", bufs=2))
            agr = ps.enter_context(tc.tile_pool(name="agr", bufs=8))
            stat = ps.enter_context(tc.tile_pool(name="stat", bufs=4))
            tgp = ps.enter_context(tc.tile_pool(name="tgp", bufs=4))
            psS = ps.enter_context(tc.tile_pool(name="psS", bufs=3, space="PSUM"))
            psC = ps.enter_context(tc.tile_pool(name="psC", bufs=4, space="PSUM"))
            psQ = ps.enter_context(tc.tile_pool(name="psQ", bufs=1, space="PSUM"))

            p0s = (g0 % 2) * BS
            p1s = (g1 % 2) * BS

            for h in range(hpc):
                r0 = h * BS
                qz, kp, va = qz_s[h % 2], kp_s[h % 2], va_s[h % 2]
                kg, vg, qg = kg_s[h % 2], vg_s[h % 2], qg_s[h % 2]

                # -- per-head DMAs (overlap previous head's compute) --
                nc.sync.dma_start(qz[0:64, :], qT_d[r0 : r0 + BS, :])
                nc.sync.dma_start(kp[0:64, BS : BS + n], kT_d[r0 : r0 + BS, :])
                nc.sync.dma_start(kp[0:64, 0:BS], kT_d[r0 : r0 + BS, n - BS : n])
                nc.sync.dma_start(kp[0:64, BS + n :], kT_d[r0 : r0 + BS, 0:BS])
                vs = v_d[:, r0 : r0 + BS]
                nc.sync.dma_start(va[0:BS, 0, 0:BS], vs[n - BS : n, :])
                nc.sync.dma_start(va[BS:P, 0, 0:BS], vs[0:BS, :])
                nc.sync.dma_start(
                    va[:, 1 : nkc - 1, 0:BS],
                    vs[BS : n - BS, :].rearrange("(a p) c -> p a c", p=P),
                )
                nc.sync.dma_start(va[0:BS, nkc - 1, 0:BS], vs[n - BS : n, :])
                nc.sync.dma_start(va[BS:P, nkc - 1, 0:BS], vs[0:BS, :])
                for gi, gv in enumerate((g0, g1)):
                    nc.sync.dma_start(
                        kg[0:64, gi * BS : (gi + 1) * BS],
                        kT_d[r0 : r0 + BS, gv * BS : (gv + 1) * BS],
                    )
                    nc.sync.dma_start(
                        vg[gi * BS : (gi + 1) * BS, 0:BS],
                        vs[gv * BS : (gv + 1) * BS, :],
                    )
                    nc.sync.dma_start(
                        qg[0:64, gi * BS : (gi + 1) * BS],
                        qT_d[r0 : r0 + BS, gv * BS : (gv + 1) * BS],
                    )

                # -- local + global-col scores (S^T layout), exp, AV --
                def sc_pair(pr):
                    """scores+exp for padded key chunks 2pr, 2pr+1 (batched)."""
                    sps = psS.tile([P, 2, 256], f32, tag="sps")
                    a_sb = apool.tile([P, 2, 256], bf16, tag="a")
                    nws = []
                    for i in (0, 1):
                        c = 2 * pr + i
                        if c > nt:
                            continue
                        lo = max(0, (c - 1)) * P
                        hi = min(nt, c + 1) * P
                        nws.append(hi - lo)
                        nc.tensor.matmul(
                            sps[:, i, 0 : hi - lo],
                            kp[:, c * P : (c + 1) * P],
                            qz[:, lo:hi],
                            start=True,
                            stop=True,
                        )
                    if nws == [256, 256]:
                        nc.scalar.activation(a_sb, sps, AF.Exp)
                    else:
                        for i, nw in enumerate(nws):
                            nc.scalar.activation(
                                a_sb[:, i, 0:nw], sps[:, i, 0:nw], AF.Exp
                            )
                    return a_sb

                def gc_group(j):
                    spg = psS.tile([P, 512], f32, tag="sps")
                    nc.tensor.matmul(
                        spg, kg, qz[:, j * 512 : (j + 1) * 512], start=True, stop=True
                    )
                    ag = agp.tile([P, 512], bf16, tag="ag")
                    nc.scalar.activation(ag, spg, AF.Exp)
                    return ag

                ag_nxt = gc_group(0)
                a_pair = {0: sc_pair(0), 1: sc_pair(1)}
                ag_cur = None
                cps2 = None
                for t in range(nt):
                    if t % 4 == 0:
                        ag_cur = ag_nxt
                    if t % 4 == 1 and t // 4 + 1 < 8:
                        ag_nxt = gc_group(t // 4 + 1)
                    want = min(nt // 2, t // 2 + 2)
                    if want not in a_pair:
                        a_pair[want] = sc_pair(want)
                        a_pair.pop(want - 3, None)
                    a_lo = a_pair[t // 2][:, t % 2, :]
                    off = 0 if t == 0 else P
                    a_up = a_pair[(t + 1) // 2][:, (t + 1) % 2, :]
                    if t % 2 == 0:
                        cps2 = psC.tile([P, 260], f32, tag="cps")
                    co = (t % 2) * 130
                    cps = cps2[:, co : co + 130]
                    nc.tensor.matmul(
                        cps[:, 0:65],
                        a_lo[:, off : off + P],
                        va[:, t, :],
                        start=True,
                        stop=False,
                    )
                    nc.tensor.matmul(
                        cps[:, 0:65],
                        a_up[:, 0:P],
                        va[:, t + 1, :],
                        start=False,
                        stop=True,
                    )
                    nc.tensor.matmul(
                        cps[:, 65:130],
                        ag_cur[:, (t % 4) * P : (t % 4 + 1) * P],
                        vg,
                        start=True,
                        stop=True,
                    )
                    if t % 2 == 0:
                        continue
                    # batched per-partition reciprocals for both tiles
                    r4 = stat.tile([P, 4], f32, tag="r4")
                    nc.vector.reciprocal(r4, cps2[:, 64:260:65])
                    for tt, cc, ri in ((t - 1, 0, 0), (t, 130, 2)):
                        tg = tgp.tile([P, BS], f32, tag="tg")
                        nc.vector.tensor_scalar_mul(
                            tg, cps2[:, cc + 65 : cc + 129], r4[:, ri + 1 : ri + 2]
                        )
                        nc.vector.scalar_tensor_tensor(
                            ctx_nat[:, tt, r0 : r0 + BS],
                            cps2[:, cc : cc + 64],
                            r4[:, ri : ri + 1],
                            tg,
                            OP.mult,
                            OP.add,
                        )

                # -- global rows: full attention for the 2 global q blocks --
                cpr0 = psQ.tile([P, 130], f32, tag="cpr0")
                cpr1 = cpr0

                def grow_scores(j):
                    spr = psS.tile([P, 4, P], f32, tag="sps")
                    for i in range(4):
                        c = 1 + 4 * j + i
                        nc.tensor.matmul(
                            spr[:, i, :],
                            kp[:, c * P : (c + 1) * P],
                            qg,
                            start=True,
                            stop=True,
                        )
                    ar = agr.tile([P, 4, P], bf16, tag="ar")
                    nc.scalar.activation(ar, spr, AF.Exp)
                    return ar

                ars = [grow_scores(0)]
                for j in range(8):
                    if j + 1 < 8:
                        ars.append(grow_scores(j + 1))
                    for i in range(4):
                        c = 1 + 4 * j + i
                        nc.tensor.matmul(
                            cpr0[p0s : p0s + BS, 0:65],
                            ars[j][:, i, 0:BS],
                            va[:, c, :],
                            start=(c == 1),
                            stop=(c == nkc - 1),
                        )
                for j in range(8):
                    for i in range(4):
                        c = 1 + 4 * j + i
                        nc.tensor.matmul(
                            cpr1[p1s : p1s + BS, 65:130],
                            ars[j][:, i, BS:P],
                            va[:, c, :],
                            start=(c == 1),
                            stop=(c == nkc - 1),
                        )
                for gi, gv, pb, co in ((0, g0, p0s, 0), (1, g1, p1s, 65)):
                    rg = stat.tile([P, 1], f32, tag=f"rg{gi}")
                    nc.vector.reciprocal(
                        rg[pb : pb + BS, :], cpr0[pb : pb + BS, co + 64 : co + 65]
                    )
                    nc.vector.tensor_scalar_mul(
                        ctx_nat[pb : pb + BS, gv // 2, r0 : r0 + BS],
                        cpr0[pb : pb + BS, co : co + 64],
                        rg[pb : pb + BS, :],
                    )

        # ---------------- pass C: output projection ----------------
        with ExitStack() as ps:
            wop = ps.enter_context(tc.tile_pool(name="wop", bufs=1))
            wo_sb = wop.tile([P, ndc2, dmodel], bf16)
            nc.sync.dma_start(wo_sb, wo_d.rearrange("(a p) o -> p a o", p=P))
            ctp = ps.enter_context(tc.tile_pool(name="ctp", bufs=2))
            copool = ps.enter_context(tc.tile_pool(name="co", bufs=4))
            psT = ps.enter_context(tc.tile_pool(name="psT", bufs=4, space="PSUM"))
            psO = ps.enter_context(tc.tile_pool(name="psO", bufs=4, space="PSUM"))
            for ncc in range(nch):
                ctxT = ctp.tile([P, ndc2, 512], bf16, tag="ctxT")
                for ti in range(4):
                    t = ncc * 4 + ti
                    for dc in range(ndc2):
                        tp = psT.tile([P, P], bf16, tag="tp")
                        nc.tensor.transpose(
                            tp, ctx_nat[:, t, dc * P : (dc + 1) * P], ident
                        )
                        if dc % 2 == 0:
                            nc.scalar.copy(ctxT[:, dc, ti * P : (ti + 1) * P], tp)
                        else:
                            nc.vector.tensor_copy(ctxT[:, dc, ti * P : (ti + 1) * P], tp)
                for ot in range(dmodel // P):
                    pp = psO.tile([P, 512], f32, tag="pso")
                    for dc in range(ndc2):
                        nc.tensor.matmul(
                            pp,
                            wo_sb[:, dc, ot * P : (ot + 1) * P],
                            ctxT[:, dc, :],
                            start=(dc == 0),
                            stop=(dc == ndc2 - 1),
                        )
                    ob = copool.tile([P, 512], bf16, tag="ob")
                    if ot % 2 == 0:
                        nc.scalar.copy(ob, pp)
                    else:
                        nc.vector.tensor_copy(ob, pp)
                    nc.sync.dma_start(
                        out_d[ot * P : (ot + 1) * P, ncc * 512 : (ncc + 1) * 512], ob
                    )

        if dbg:
            nc.sync.dma_start(qTo_d[:, :], qT_d)
            nc.sync.dma_start(kTo_d[:, :], kT_d)
            nc.sync.dma_start(vo_d[:, :], v_d)
            nc.sync.dma_start(ctxo_d[:, :, :], ctx_nat)

    nc.finalize()
    return nc


@functools.lru_cache(maxsize=8)
def _get(n, dmodel, dl, g0, g1):
    return _build(n, dmodel, dl, g0, g1)


def _prepare(inputs):
    """Build (nc, in_maps, meta) for the SPMD run from full unsharded inputs."""
    x = np.asarray(inputs["x"], np.float32)
    Wq = np.asarray(inputs["Wq"], np.float32)
    Wk = np.asarray(inputs["Wk"], np.float32)
    Wv = np.asarray(inputs["Wv"], np.float32)
    Wo = np.asarray(inputs["Wo"], np.float32)
    bq = np.asarray(inputs["bq"], np.float32)
    bk = np.asarray(inputs["bk"], np.float32)
    bv = np.asarray(inputs["bv"], np.float32)
    bo = np.asarray(inputs["bo"], np.float32)
    gi = np.asarray(inputs["global_indices"]).astype(np.int64)
    g0, g1 = int(gi[0]), int(gi[1])

    b_, n_, d_ = x.shape
    dl = d_ // 2
    scale = 1.0 / np.sqrt(np.float32(64.0)).astype(np.float32)

    nc = _get(n_, d_, dl, g0, g1)

    import ml_dtypes

    bf = ml_dtypes.bfloat16
    # mask pattern rows (periodic in the column index, see pass B docstring)
    NEGf = np.float32(-1e9)
    j = np.arange(n_) % 256
    qmask = np.zeros((64, n_), np.float32)
    qmask[0, (j >= 128) & (j < 192)] = 1.0  # w1e
    qmask[1, (j >= 64) & (j < 128)] = 1.0   # w2e
    qmask[2, j < 64] = 1.0                  # w1o
    qmask[3, j >= 192] = 1.0                # w2o
    qmask = np.ascontiguousarray(qmask).astype(bf)
    jk = np.arange(n_ + 128) % 256
    kmask = np.zeros((64, n_ + 128), np.float32)
    kmask[0, (jk >= 64) & (jk < 128)] = NEGf   # u1e
    kmask[1, jk < 64] = NEGf                   # u2e
    kmask[2, jk >= 192] = NEGf                 # u1o
    kmask[3, (jk >= 128) & (jk < 192)] = NEGf  # u2o
    kmask = np.ascontiguousarray(kmask).astype(bf)
    in_maps = []
    for c in range(8):
        b, hg = divmod(c, 2)
        S = slice(hg * dl, (hg + 1) * dl)
        in_maps.append(
            {
                "xT": np.ascontiguousarray(x[b].T).astype(bf),
                "qmask": qmask,
                "kmask": kmask,
                "wqT": np.ascontiguousarray((Wq[S, :] * scale).T).astype(bf),
                "wkT": np.ascontiguousarray(Wk[S, :].T).astype(bf),
                "wvT": np.ascontiguousarray(Wv[S, :].T).astype(bf),
                "woT": np.ascontiguousarray(Wo[:, S].T).astype(bf),
                "bq": np.ascontiguousarray(bq[S] * scale),
                "bk": np.ascontiguousarray(bk[S]),
            }
        )

    # host-side bv correction: out[q] += c(q) * bv @ Wo.T, c(q)=1 on global
    # blocks (overwritten by full-attention rows), else 2.
    bvWo = bv @ Wo.T  # [d_model]
    coef = np.full((n_, 1), 2.0, np.float32)
    bs = 64
    coef[g0 * bs : (g0 + 1) * bs] = 1.0
    coef[g1 * bs : (g1 + 1) * bs] = 1.0
    corr = (coef * bvWo[None, :] + bo[None, :]).astype(np.float32)

    return nc, in_maps, (b_, n_, d_, corr)


def _combine(res, meta):
    b_, n_, d_, corr = meta
    out = np.empty((b_, n_, d_), np.float32)
    for b in range(b_):
        out[b] = (
            res[2 * b]["outT"].T.astype(np.float32)
            + res[2 * b + 1]["outT"].T.astype(np.float32)
            + corr
        )
    return out


def kernel(**inputs):
    _ensure_path()
    from concourse.bass_utils import run_bass_kernel_spmd

    nc, in_maps, meta = _prepare(inputs)
    res = run_bass_kernel_spmd(nc, in_maps, list(range(8))).results
    return _combine(res, meta)
